# revision 1
# baseline (speedup 1.0000x reference)
"""Trainium2 Bass kernel for nn_Eq2NetSet (pairwise relu-MLP + mean pool + decode).

Reference computation (B=4, N=256, D=64, H=128):
    x[b,i,:] = concat(emb[xcat[b,i]], xfeat[b,i])            # [B,N,64]
    p[b,i,j,:] = x[b,i,:] * x[b,j,:]                          # elementwise
    h = relu(relu(relu(p@W1+b1)@W2+b2)@W3+b3)                 # [B,N,N,128]
    pooled = relu(mean_{i,j} h)                               # [B,128]
    out = relu(relu(pooled@D1+c1)@D2+c2)@D3+c3                # [B,1]

Symmetry: p[b,i,j]==p[b,j,i], so each (b,i) row only evaluates the half-circle
j in {i..i+127 mod N}: every unordered pair {i,j} with circular distance 1..127
is covered exactly once, the diagonal once, distance-128 pairs not at all.
With S_cov = sum over covered (i,j), S_diag, S_128 (host-computed, tiny):
full-grid sum = 2*S_cov - S_diag + S_128.

Sharding: 8 cores = 4 batches x 2 row-halves.  Each core gets a rotated
wrap-extended transposed feature matrix stacked twice on the partition axis
with a one-column shift (bf16):
  xe[0:64,  t] = X[b].T[:, (base+t)   % 256]
  xe[64:128,t] = X[b].T[:, (base+t+1) % 256]
so local rows k / k+1 share one tensor_scalar (p-gen).  An fp32 sidecar `xs`
holds the even-row scalar columns (TensorScalarPtr requires fp32 scalars).

Engine economy (the binding constraint is PSUM->SBUF relu evacuation, which
only Act (1.2 col/ns) and DVE (0.96 col/ns) can perform; GPSIMD has no PSUM
port):
  - p-gen runs on GPSIMD (SBUF->SBUF, otherwise idle), freeing the DVE.
  - h1/h2 evac columns are split Act/DVE (A1, A2 knobs).
  - h3 evac is Act with fused accum_out (sum over pairs).
  - matmuls are bf16; L1 runs as two 64-row-tile matmuls (tile (0,0)/(64,0)).
    (K_SPLITK=1 would run L2/L3 as accumulating K=64 pairs — it doubles PE
    time AND crashes on HW via cross-tile same-bank PSUM accumulation; keep 0.)
  - all weights arrive in ONE packed DMA; xe in 2 chunks; a dummy activation
    at t=0 pulls the ~2.7us act-table load under the DMA shadow.
"""

import os
import sys

import numpy as np

sys.path.insert(0, "/opt/trn_rl_repo")

B, N, D, H = 4, 256, 64, 128
NCORES = 8
ROWS = 128          # i-rows per core
JS = 128            # j-window per row (half circle)
RPI = 8             # rows folded into one iteration
FREE = RPI * JS     # matmul-output columns per iteration (1024)
NIT = ROWS // RPI   # 16

# HW-benched: ScalarE ops are ~2x costlier than modeled (ScE errata), so
# keep the Act share of h1/h2 evac small: marginal 54.2us @A=432,
# 50.6 @A=288, 44.4 @A=160 (A=0 also tested; see git/bench logs).
A1 = int(os.environ.get("K_A1", "160"))     # h1 cols on Act (rest DVE)
A2 = int(os.environ.get("K_A2", "160"))     # h2 cols on Act (rest DVE)
# GPSIMD tensor_scalar measured ~1.5us/op on HW (slow Q7 ucode path) vs
# ~0.1us on DVE bf16 4x — keep p-gen on the DVE (PGP=0).
PGP = int(os.environ.get("K_PGP", "0"))     # of 4 p-gen ops, how many on Pool
SPLITK = bool(int(os.environ.get("K_SPLITK", "0")))  # L2/L3 as 2x64-row tiles
PBUFS = int(os.environ.get("K_PBUFS", "4"))
HBUFS = int(os.environ.get("K_HBUFS", "4"))
PSB = tuple(int(x) for x in os.environ.get("K_PSB", "1,2,1").split(","))
SKEW = bool(int(os.environ.get("K_SKEW", "1")))  # software-pipelined emission
XDMA0 = int(os.environ.get("K_XDMA0", "160"))  # first xe chunk columns
HWLOOP = int(os.environ.get("K_HWLOOP", "0"))  # >0: wrap main loop in For_i
UNROLL = int(os.environ.get("K_UNROLL", "1"))  # bodies per For_i iteration

_STATE: dict = {}

# Set by test.py to capture a profiled run.
PROFILE = bool(int(os.environ.get("KERNEL_PROFILE", "0")))
LAST_EXEC_TIME_NS = None


def _build_program():
    import concourse.mybir as mybir
    import concourse.tile as tile
    from concourse import bacc

    f32 = mybir.dt.float32
    bf16 = mybir.dt.bfloat16
    Relu = mybir.ActivationFunctionType.Relu
    op_add = mybir.AluOpType.add
    op_max = mybir.AluOpType.max
    ax_x = mybir.AxisListType.X

    nc = bacc.Bacc("TRN2", target_bir_lowering=False)

    # xin cols 0:134 = bit-packed fp32 [128,67]: 64 even-row scalars + b1|b2|b3
    #     cols 134:519 = xe (bf16 rotated/stacked features, 385 cols)
    XOFF = 134
    xin_d = nc.dram_tensor("xin", [128, 520], bf16, kind="ExternalInput")
    # packed params: cols 0:128 w1 (dup'd), 128:256 w2, 256:384 w3
    par_d = nc.dram_tensor("par", [128, 384], bf16, kind="ExternalInput")
    out_d = nc.dram_tensor("out", [128, 1], f32, kind="ExternalOutput")

    with tile.TileContext(nc) as tc:
        with (
            tc.tile_pool(name="singles", bufs=1) as singles,
            tc.tile_pool(name="p", bufs=PBUFS) as p_pool,
            tc.tile_pool(name="h1", bufs=HBUFS) as h1_pool,
            tc.tile_pool(name="h2", bufs=HBUFS) as h2_pool,
            tc.tile_pool(name="h3", bufs=2) as h3_pool,
            tc.tile_pool(name="ps1", bufs=PSB[0], space="PSUM") as ps1_pool,
            tc.tile_pool(name="ps2", bufs=PSB[1], space="PSUM") as ps2_pool,
            tc.tile_pool(name="ps3", bufs=PSB[2], space="PSUM") as ps3_pool,
        ):
            xin = singles.tile([128, 520], bf16)
            par = singles.tile([128, 384], bf16)
            acc = singles.tile([128, NIT], f32)
            outs = singles.tile([128, 1], f32)
            dum = singles.tile([128, 1], f32)

            # Pull the activation-table load to t=0 (overlaps input DMAs).
            nc.vector.memset(dum[:, :], 0.0)
            nc.scalar.activation(out=dum[:, :], in_=dum[:, :], func=Relu)

            # Input DMAs, critical-path order: the first p-gen needs the
            # scalars + xe[:, :XDMA0]; the first matmul needs w1; w2/w3 and
            # the rest of xe are only needed a few pipeline steps later.
            # xin rides the SP HWDGE ring, par the ACT ring (parallel gen);
            # the ACT sequencer configures the par DMAs while its engine is
            # busy with the table load.
            c0 = XOFF + XDMA0
            if XDMA0 >= 385:
                nc.sync.dma_start(out=xin[:, :], in_=xin_d[:, :])
                nc.scalar.dma_start(out=par[:, :], in_=par_d[:, :])
            else:
                nc.sync.dma_start(out=xin[:, 0:c0], in_=xin_d[:, 0:c0])
                nc.scalar.dma_start(out=par[:, 0:128], in_=par_d[:, 0:128])
                nc.scalar.dma_start(out=par[:, 128:384], in_=par_d[:, 128:384])
                nc.sync.dma_start(out=xin[:, c0:520], in_=xin_d[:, c0:520])

            xe = xin[:, XOFF:XOFF + 385]
            xs = xin[:, 0:XOFF].bitcast(f32)  # [128, 67]
            w1 = par[:, 0:128]
            w2 = par[:, 128:256]
            w3 = par[:, 256:384]
            b1 = xs[:, 64:65]
            b2 = xs[:, 65:66]
            b3 = xs[:, 66:67]

            def layer23(ps_out_pool, w, h_in, tag):
                """K=128 matmul (ps = w.T @ h_in) over FREE cols."""
                ps = ps_out_pool.tile([128, FREE], f32, tag=tag)
                for c in range(0, FREE, 512):
                    e = c + 512
                    if SPLITK:
                        nc.tensor.matmul(
                            ps[:, c:e], w[0:64, :], h_in[0:64, c:e],
                            start=True, stop=False, tile_position=(0, 0))
                        nc.tensor.matmul(
                            ps[:, c:e], w[64:128, :], h_in[64:128, c:e],
                            start=False, stop=True, tile_position=(64, 0))
                    else:
                        nc.tensor.matmul(ps[:, c:e], w[:, :], h_in[:, c:e])
                return ps

            def evac(ps, h_t, bias, a_cols):
                """relu(ps + bias) -> h_t (bf16), split Act/DVE at a_cols."""
                if a_cols > 0:
                    nc.scalar.activation(
                        out=h_t[:, 0:a_cols], in_=ps[:, 0:a_cols],
                        func=Relu, bias=bias)
                if a_cols < FREE:
                    nc.vector.tensor_scalar(
                        out=h_t[:, a_cols:FREE], in0=ps[:, a_cols:FREE],
                        scalar1=bias, scalar2=0.0, op0=op_add, op1=op_max)

            def pgen(k):
                row0 = RPI * k
                p_t = p_pool.tile([128, FREE // 2], bf16, tag="p")
                for s in range(RPI // 2):
                    r = row0 + 2 * s
                    # k<2: DVE (idle during startup; Pool's 4 serial ops
                    # would delay the first matmul by ~1.4us)
                    eng = nc.gpsimd if (s < PGP and k >= 2) else nc.vector
                    eng.tensor_scalar_mul(
                        out=p_t[:, s * JS:(s + 1) * JS],
                        in0=xe[:, r:r + JS],
                        scalar1=xs[:, r // 2:r // 2 + 1],
                    )
                return p_t

            def stage1(p_t):
                # L1: two 64-row-tile matmuls (T0 even rows, T8 odd rows).
                ps1 = ps1_pool.tile([128, FREE], f32, tag="ps1")
                nc.tensor.matmul(
                    ps1[:, 0:FREE // 2], w1[0:64, :], p_t[0:64, :],
                    tile_position=(0, 0))
                nc.tensor.matmul(
                    ps1[:, FREE // 2:FREE], w1[64:128, :], p_t[64:128, :],
                    tile_position=(64, 0))
                h1t = h1_pool.tile([128, FREE], bf16, tag="h1")
                evac(ps1, h1t, b1, A1)
                return h1t

            def stage2(h1t):
                ps2 = layer23(ps2_pool, w2, h1t, "ps2")
                h2t = h2_pool.tile([128, FREE], bf16, tag="h2")
                evac(ps2, h2t, b2, A2)
                return h2t

            def stage3(h2t, k):
                ps3 = layer23(ps3_pool, w3, h2t, "ps3")
                h3t = h3_pool.tile([128, FREE], bf16, tag="h3")
                nc.scalar.activation(
                    out=h3t[:, :], in_=ps3[:, :], func=Relu, bias=b3,
                    accum_out=acc[:, k:k + 1])

            def main_loop():
                if not SKEW:
                    for k in range(NIT):
                        stage3(stage2(stage1(pgen(k))), k)
                    return
                # Software-pipelined: per step k emit pgen(k+1), L1(k),
                # evac1(k), L2(k-1), evac2(k-1), L3(k-2), h3(k-2) so no
                # engine queue head-of-line-blocks on the previous stage.
                # DVE queue per step: [D1(k), pgen(k+1), D2(k-1)] — D1 first
                # cuts h1-evac latency; pgen still lands a step ahead of its
                # consumer L1(k+1).
                pq, h1q, h2q = {}, {}, {}
                pq[0] = pgen(0)
                for k in range(NIT + 2):
                    if k < NIT:
                        h1q[k] = stage1(pq.pop(k))
                    if k + 1 < NIT:
                        pq[k + 1] = pgen(k + 1)
                    if 0 <= k - 1 < NIT:
                        h2q[k - 1] = stage2(h1q.pop(k - 1))
                    if 0 <= k - 2 < NIT:
                        stage3(h2q.pop(k - 2), k - 2)

            if HWLOOP > 0:
                with tc.For_i(0, HWLOOP):
                    for _ in range(UNROLL):
                        main_loop()
            else:
                main_loop()

            nc.vector.tensor_reduce(
                out=outs[:, 0:1], in_=acc[:, :], axis=ax_x, op=op_add)
            nc.sync.dma_start(out=out_d[:, :], in_=outs[:, :])

    nc.compile()
    return nc


def _get_state():
    if "nc" not in _STATE:
        _STATE["nc"] = _build_program()
    return _STATE


def make_in_maps(inputs):
    xcat = np.asarray(inputs["xcat"])
    xfeat = np.asarray(inputs["xfeat"], dtype=np.float32)
    emb = np.asarray(inputs["emb"], dtype=np.float32)
    W1 = np.asarray(inputs["W1"], dtype=np.float32)
    W1d = np.concatenate([W1, W1], axis=0)  # [128,128]
    W2 = np.asarray(inputs["W2"], dtype=np.float32)
    W3 = np.asarray(inputs["W3"], dtype=np.float32)
    par = np.ascontiguousarray(
        np.concatenate([W1d, W2, W3], axis=1)).astype(np.float32)
    par_bf = _to_bf16(par)
    bias = np.stack(
        [np.asarray(inputs["b1"], np.float32),
         np.asarray(inputs["b2"], np.float32),
         np.asarray(inputs["b3"], np.float32)], axis=1)  # [128, 3]

    # x = concat(emb[xcat], xfeat[...,None]) -> [B,N,D]
    X = np.concatenate(
        [emb[xcat], xfeat[..., None]], axis=-1).astype(np.float32)

    idx = np.arange(385)
    in_maps = []
    for c in range(NCORES):
        b, half = divmod(c, 2)
        base = half * ROWS
        xt = X[b].T  # [64, 256]
        top = xt[:, (base + idx) % N]
        bot = xt[:, (base + idx + 1) % N]
        xe = np.ascontiguousarray(
            np.concatenate([top, bot], axis=0), dtype=np.float32)  # [128,385]
        xe_bf = _to_bf16(xe)
        # fp32 scalar sidecar: col j = xe[:, 2j] (matching the bf16-rounded
        # xe), with the three bias columns appended; bit-packed as bf16 pairs
        # in front of xe.
        import ml_dtypes
        xs = np.ascontiguousarray(np.concatenate(
            [_from_bf16(xe_bf[:, 0:128:2]).astype(np.float32), bias], axis=1))
        xin = np.ascontiguousarray(np.concatenate(
            [xs.view(ml_dtypes.bfloat16), xe_bf,
             np.zeros((128, 1), ml_dtypes.bfloat16)], axis=1))  # [128, 520]
        in_maps.append({"xin": xin, "par": par_bf})
    return in_maps


def _to_bf16(a):
    import ml_dtypes
    return np.ascontiguousarray(a.astype(ml_dtypes.bfloat16))


def _from_bf16(a):
    return a.astype(np.float32)


def _host_extra_terms(inputs):
    """Per-batch S_diag and S_128 (both-orientations) computed on host."""
    emb = np.asarray(inputs["emb"], dtype=np.float32)
    X = np.concatenate(
        [emb[np.asarray(inputs["xcat"])],
         np.asarray(inputs["xfeat"], dtype=np.float32)[..., None]],
        axis=-1).astype(np.float32)
    W1 = np.asarray(inputs["W1"], np.float32)
    W2 = np.asarray(inputs["W2"], np.float32)
    W3 = np.asarray(inputs["W3"], np.float32)
    b1 = np.asarray(inputs["b1"], np.float32)
    b2 = np.asarray(inputs["b2"], np.float32)
    b3 = np.asarray(inputs["b3"], np.float32)
    sdiag = np.zeros((B, H), np.float32)
    s128 = np.zeros((B, H), np.float32)
    for b in range(B):
        pd = X[b] * X[b]                                   # [256, 64]
        p8 = X[b][:128] * X[b][128:]                       # [128, 64]
        for p, dst, w in ((pd, sdiag, 1.0), (p8, s128, 2.0)):
            h = np.maximum(p @ W1 + b1, 0.0)
            h = np.maximum(h @ W2 + b2, 0.0)
            h = np.maximum(h @ W3 + b3, 0.0)
            dst[b] = w * h.sum(axis=0)
    return sdiag, s128


def combine_outputs(outs, inputs):
    pooled = np.zeros((B, H), dtype=np.float32)
    sdiag_h, s128_h = _host_extra_terms(inputs)
    for b in range(B):
        oe = outs[2 * b].astype(np.float32)
        oo = outs[2 * b + 1].astype(np.float32)
        s_cov = oe[:, 0] + oo[:, 0]
        full = 2.0 * s_cov - sdiag_h[b] + s128_h[b]
        pooled[b] = np.maximum(full / np.float32(N * N), 0.0)

    D1 = np.asarray(inputs["D1"], dtype=np.float32)
    c1 = np.asarray(inputs["c1"], dtype=np.float32)
    D2 = np.asarray(inputs["D2"], dtype=np.float32)
    c2 = np.asarray(inputs["c2"], dtype=np.float32)
    D3 = np.asarray(inputs["D3"], dtype=np.float32)
    c3 = np.asarray(inputs["c3"], dtype=np.float32)

    h = np.maximum(pooled @ D1 + c1, 0.0)
    h = np.maximum(h @ D2 + c2, 0.0)
    return (h @ D3 + c3).astype(np.float32)


def kernel(**inputs) -> np.ndarray:
    global LAST_EXEC_TIME_NS
    from concourse.bass_utils import run_bass_kernel_spmd

    st = _get_state()
    nc = st["nc"]
    in_maps = make_in_maps(inputs)

    kwargs = {}
    if PROFILE:
        kwargs = dict(trace=True, trace_cores=list(range(NCORES)))
    try:
        res = run_bass_kernel_spmd(
            nc, in_maps, core_ids=list(range(NCORES)), **kwargs)
    except (ImportError, ModuleNotFoundError):
        # NTFF profiling hook unavailable in this container; run untraced.
        res = run_bass_kernel_spmd(nc, in_maps, core_ids=list(range(NCORES)))
    if PROFILE:
        LAST_EXEC_TIME_NS = res.exec_time_ns
        _STATE["last_result"] = res

    outs = [r["out"] for r in res.results]  # each [128, 1]
    return combine_outputs(outs, inputs)

